# revision 32
# baseline (speedup 1.0000x reference)
"""Trainium2 Bass kernel for GQA attention with RoPE (causal), sharded
TP4 x DP2 across 8 NeuronCores.

Reference computation (all fp32):
  q = (x @ wq.T)  -> [B,S,16,128], k/v = (x @ wk/wv.T) -> [B,S,4,128]
  q,k roped with interleaved-pair rotation; repeat_kv(4); causal softmax(qk/sqrt(128)) @ v
  out = attn @ wo.T

Sharding: core i handles batch i//4 and head-group g=i%4: q heads
{4g..4g+3} and kv head g (exactly the kv head those q heads attend to).
wq/wk/wv are column-sharded 4-way (no kv redundancy), wo row-sharded;
the all-reduce over the 4 partial wo outputs per batch happens on the host.

All matmul operands are fp16 (PSUM accumulation stays fp32); x / weights /
y partials travel as fp16, halving HBM traffic. Host-side prep (layout
only): x pre-transposed to [D,S] per batch; weights repacked ko-major so
chunk 0 can stream them; the head_dim axis of wq/wk is permuted so RoPE
pairs are de-interleaved (real in rows 0..63, imag in 64..127 per head),
turning RoPE into ops on contiguous 64-partition slices (the score
contraction q.k is invariant to this permutation).

Structure per core:
  proj chunk 0: ko-outer over 6 PSUM accumulators (q0..q3,k,v) so compute
        consumes x/w tiles in exactly their DMA arrival order -- the
        startup window is aggregate-DMA-bound (~5MB across 3 ~100GB/s
        queues) and any other order stalls the PE. Chunks 1-3: f-outer
        with the full 8-bank pool for drain slack; x prefetched one chunk
        ahead. v is transposed to natural [s,e] by the DMA XBAR.
  attention: scores^T layout [sk, sq], softmax over sk via ones-matmul
        rowsums (no max subtraction -- |scores| <~ 5); 2-deep software
        pipeline; fully-masked leading columns of diagonal blocks skipped;
        normalize = reciprocal_approx_fast + partition_broadcast + mul.
  wo: chunk-pipelined one chunk behind attention; y copies on DVE (ACT is
        exp-saturated during attention), stores rotate over all 3 queues.
"""

import math
import sys
from contextlib import ExitStack

import numpy as np

if "/opt/trn_rl_repo" not in sys.path:
    sys.path.insert(0, "/opt/trn_rl_repo")

B = 2
S = 2048
D = 2048
N_HEADS = 16
N_KV_HEADS = 4
HEAD_DIM = 128
N_CORES = 8
HPC = 4  # q heads per core
NF = HPC + 2  # per-core projection feature blocks: q0..q3, k, v
SC = 512  # sequence chunk (matmul moving free dim)
NKO = D // 128  # contraction chunks for the projections = 16
NSB = S // 128  # 128-row seq blocks = 16
NCH = S // SC  # 512-wide seq chunks = 4
SCALE = 1.0 / math.sqrt(HEAD_DIM)

_CACHE = {}


def _build_module():
    import concourse.tile as tile
    from concourse import bacc, mybir

    f32 = mybir.dt.float32
    f16 = mybir.dt.float16

    nc = bacc.Bacc(
        "TRN2",
        target_bir_lowering=False,
        debug=False,
        enable_asserts=False,
        num_devices=N_CORES,
    )
    xT = nc.dram_tensor("xT", [D, S], f16, kind="ExternalInput").ap()
    wT = nc.dram_tensor("wT", [NKO, 128, NF, 128], f16, kind="ExternalInput").ap()
    woT = nc.dram_tensor("woT", [HPC * 128, D], f16, kind="ExternalInput").ap()
    cs = nc.dram_tensor("cs", [128, S], f16, kind="ExternalInput").ap()
    csw = nc.dram_tensor("csw", [128, S], f16, kind="ExternalInput").ap()
    mask = nc.dram_tensor("mask", [128, 1024], f16, kind="ExternalInput").ap()
    onesd = nc.dram_tensor("onesd", [128, 1], f16, kind="ExternalInput").ap()
    y = nc.dram_tensor("y", [S, D], f16, kind="ExternalOutput").ap()

    with tile.TileContext(nc) as tc, ExitStack() as ctx:
        consts = ctx.enter_context(tc.tile_pool(name="consts", bufs=1))
        xp = ctx.enter_context(tc.tile_pool(name="xp", bufs=3))
        qk_pool = ctx.enter_context(tc.tile_pool(name="qk", bufs=1))
        v_pool = ctx.enter_context(tc.tile_pool(name="v", bufs=1))
        oT_pool = ctx.enter_context(tc.tile_pool(name="oT", bufs=1))
        es_pool = ctx.enter_context(tc.tile_pool(name="es", bufs=5))
        rope_tmp = ctx.enter_context(tc.tile_pool(name="ropetmp", bufs=2))
        qkraw_pool = ctx.enter_context(tc.tile_pool(name="qkraw", bufs=1))
        r_pool = ctx.enter_context(tc.tile_pool(name="r", bufs=2))
        rb_pool = ctx.enter_context(tc.tile_pool(name="rb", bufs=2))
        y_pool = ctx.enter_context(tc.tile_pool(name="y", bufs=8))

        w_sb = consts.tile([128, NKO, NF, 128], f16)
        woT_sb = consts.tile([128, HPC, D], f16)
        cs_sb = consts.tile([128, S], f16)
        csw_sb = consts.tile([128, S], f16)
        mask_sb = consts.tile([128, 1024], f16)
        ones_sb = consts.tile([128, 1], f16)

        qkT = qk_pool.tile([128, NF - 1, S], f16)  # [e, {q0..q3,k}, s]
        vT_sb = v_pool.tile([128, S], f16, tag="vT")  # [e, s]
        v_sb = v_pool.tile([128, NSB, 128], f16, tag="v")  # [s_in_blk, blk, e]
        oT = oT_pool.tile([128, HPC, S], f16)  # [e, head, s]

        woT3 = woT.rearrange("(ko p) f -> p ko f", p=128)
        xT3 = xT.rearrange("(ko p) s -> p ko s", p=128)

        sy, gp, sc_ = nc.sync, nc.gpsimd, nc.scalar
        dma_engs = [sy, gp, sc_]

        xts = {}

        def load_x(j, ko, eng):
            sj = slice(SC * j, SC * (j + 1))
            eng.dma_start(xts[j][:, ko, :], xT[128 * ko : 128 * (ko + 1), sj])

        def load_x2(j, g, eng):  # 2-ko group
            sj = slice(SC * j, SC * (j + 1))
            eng.dma_start(
                xts[j][:, 2 * g : 2 * (g + 1), :], xT3[:, 2 * g : 2 * (g + 1), sj]
            )

        def load_w(ko, eng):
            eng.dma_start(w_sb[:, ko, :, :], wT[ko])

        def load_cs(j, eng):
            sj = slice(SC * j, SC * (j + 1))
            eng.dma_start(cs_sb[:, sj], cs[:, sj])
            eng.dma_start(csw_sb[:, sj], csw[:, sj])

        def drain_copies(f, ps, qk_raw, qk_swap, eng):
            """Drain proj PSUM into qk_raw=[p_r;p_i], qk_swap=[p_i;p_r] via
            fast copies on `eng` (ACT or DVE) so the pool releases without
            waiting on RoPE."""
            cp = nc.scalar.copy if eng == "act" else (
                lambda o, i: nc.vector.tensor_copy(o, i)
            )
            cp(qk_raw[:, f, :], ps[:])
            cp(qk_swap[0:64, f, :], ps[64:128, :])
            cp(qk_swap[64:128, f, :], ps[0:64, :])

        def rope_muls(f, sj, qk_raw, qk_swap):
            """All-fp16 DVE RoPE off the drained copies. cs_sb=[cos;sin],
            csw_sb=[sin;cos]. (Every SB+SB DVE operand pair shares its base
            partition, as the walrus verifier requires.)"""
            t1 = rope_tmp.tile([128, SC], f16, tag="t1")
            t2 = rope_tmp.tile([128, SC], f16, tag="t2")
            nc.vector.tensor_mul(t1[0:64, :], qk_raw[0:64, f, :], cs_sb[0:64, sj])
            nc.vector.tensor_mul(
                t1[64:128, :], qk_swap[64:128, f, :], cs_sb[64:128, sj]
            )
            nc.vector.tensor_mul(t2[0:64, :], qk_swap[0:64, f, :], csw_sb[0:64, sj])
            nc.vector.tensor_mul(
                t2[64:128, :], qk_raw[64:128, f, :], csw_sb[64:128, sj]
            )
            nc.vector.tensor_sub(qkT[0:64, f, sj], t1[0:64, :], t2[0:64, :])
            nc.vector.tensor_add(qkT[64:128, f, sj], t1[64:128, :], t2[64:128, :])

        def rope_drain_act(f, sj, ps, qk_raw, qk_swap):
            drain_copies(f, ps, qk_raw, qk_swap, "act")
            rope_muls(f, sj, qk_raw, qk_swap)

        def rope_drain_dve(f, sj, ps):
            """RoPE straight from PSUM on DVE (proj pool has drain slack)."""
            t1 = rope_tmp.tile([128, SC], f16, tag="t1")
            t2 = rope_tmp.tile([128, SC], f16, tag="t2")
            pr = ps[0:64, :]
            pi = ps[64:128, :]
            nc.vector.tensor_mul(t1[0:64, :], pr, cs_sb[0:64, sj])
            nc.vector.tensor_mul(t1[64:128, :], pr, cs_sb[64:128, sj])
            nc.vector.tensor_mul(t2[0:64, :], pi, cs_sb[64:128, sj])
            nc.vector.tensor_mul(t2[64:128, :], pi, cs_sb[0:64, sj])
            nc.vector.tensor_sub(qkT[0:64, f, sj], t1[0:64, :], t2[0:64, :])
            nc.vector.tensor_add(qkT[64:128, f, sj], t1[64:128, :], t2[64:128, :])

        def drain_v(j, ps):
            # v: psum -> sbuf [e,s], then one DMA-XBAR transpose to natural
            # [s_in_blk, blk, e] (no PSUM/PE involvement)
            sj = slice(SC * j, SC * (j + 1))
            nc.scalar.copy(vT_sb[:, sj], ps[:])
            (sy if j % 2 == 0 else sc_).dma_start(
                v_sb[:, 4 * j : 4 * (j + 1), :], vT_sb[:, sj], transpose=True
            )

        deferred_rope = []

        # ---- projections: qT/kT in [e, s] layout (roped), v DMA-transposed ----
        # bufs=6 (chunk-0's ko-outer needs exactly 6) leaves 2 PSUM banks
        # permanently free so the attention pools can start allocating
        # before chunk 3's drains complete
        with tc.tile_pool(name="ps_proj", bufs=6, space="PSUM") as ps_proj:
            # -- chunk 0: ko-outer, paced to the startup DMA stream --
            xts[0] = xp.tile([128, NKO, SC], f16, tag="x", name="xt0")
            xts[1] = xp.tile([128, NKO, SC], f16, tag="x", name="xt1")
            # startup burst: (w[ko], x0[ko]) pairs round-robin over the 3
            # queues in consumption order, consts and chunk-1 x at the tail;
            # w[ko=0] split in halves so the very first matmul starts sooner
            for ko in range(NKO):
                if ko == 0:
                    sy.dma_start(w_sb[:, 0, 0:3, :], wT[0][:, 0:3, :])
                    gp.dma_start(xts[0][:, 0, :], xT[0:128, 0:SC])
                    sc_.dma_start(w_sb[:, 0, 3:6, :], wT[0][:, 3:6, :])
                    continue
                load_w(ko, dma_engs[(2 * ko) % 3])
                load_x(0, ko, dma_engs[(2 * ko + 1) % 3])
                if ko == 4:
                    load_cs(0, dma_engs[2 * ko % 3])
            sc_.dma_start(mask_sb[:], mask)
            sc_.dma_start(ones_sb[:], onesd)
            for g in range(8):
                load_x2(1, g, dma_engs[g % 3])
            load_cs(1, sc_)

            ps0 = [ps_proj.tile([128, SC], f32, tag="p", name=f"ps0_{f}") for f in range(NF)]
            for ko in range(NKO):
                for f in range(NF):
                    nc.tensor.matmul(
                        ps0[f][:],
                        w_sb[:, ko, f, :],
                        xts[0][:, ko, :],
                        start=ko == 0,
                        stop=ko == NKO - 1,
                    )
            qk_raw = qkraw_pool.tile([128, NF - 1, SC], f16, tag="qkraw")
            qk_swap = qkraw_pool.tile([128, NF - 1, SC], f16, tag="qkswap")
            for f in range(NF - 1):
                rope_drain_act(f, slice(0, SC), ps0[f], qk_raw, qk_swap)
            drain_v(0, ps0[NF - 1])

            # -- chunks 1..3: f-outer with drain slack --
            for j in range(1, NCH):
                sj = slice(SC * j, SC * (j + 1))
                if j < NCH - 1:
                    xts[j + 1] = xp.tile(
                        [128, NKO, SC], f16, tag="x", name=f"xt{j+1}"
                    )
                    for g in range(8):
                        load_x2(j + 1, g, dma_engs[(g + j) % 3])
                    load_cs(j + 1, sc_)
                if j == 1:
                    for ko in range(2):
                        sc_.dma_start(
                            woT_sb[:, 2 * ko : 2 * ko + 2, :],
                            woT3[:, 2 * ko : 2 * ko + 2, :],
                        )
                last = j == NCH - 1
                if last:
                    qk_raw = qkraw_pool.tile([128, NF - 1, SC], f16, tag="qkraw")
                    qk_swap = qkraw_pool.tile([128, NF - 1, SC], f16, tag="qkswap")
                for f in range(NF):
                    ps = ps_proj.tile([128, SC], f32, tag="p")
                    for ko in range(NKO):
                        nc.tensor.matmul(
                            ps[:],
                            w_sb[:, ko, f, :],
                            xts[j][:, ko, :],
                            start=ko == 0,
                            stop=ko == NKO - 1,
                        )
                    if f == NF - 1:
                        drain_v(j, ps)
                    elif last:
                        # split drain copies across ACT and DVE so neither
                        # engine's queue delays the attention phase start;
                        # the RoPE muls are deferred until after attn(c0)
                        # (chunk-3 qkT is only read by attn(c3))
                        drain_copies(f, ps, qk_raw, qk_swap, "act" if f % 2 else "dve")
                        deferred_rope.append((f, sj, qk_raw, qk_swap))
                    else:
                        rope_drain_dve(f, sj, ps)

        # ---- attention (scores^T layout: [sk, sq]; softmax over sk via
        # ones-matmul rowsums; no max subtraction -- |scores| <~ 5) ----
        with (
            tc.tile_pool(name="ps_attn_s", bufs=3, space="PSUM") as ps_attn_s,
            tc.tile_pool(name="ps_attn_o", bufs=2, space="PSUM") as ps_attn_o,
            tc.tile_pool(name="ps_attn_r", bufs=1, space="PSUM") as ps_attn_r,
            tc.tile_pool(name="ps_y", bufs=2, space="PSUM") as ps_yp,
        ):

            def emit_wo(c):
                # output projection for s-chunk c: y[s,f] = sum_e oT[e,s]*woT[e,f]
                for m4 in range(4):
                    m = 4 * c + m4
                    for fc in range(NCH):
                        fj = slice(SC * fc, SC * (fc + 1))
                        ps_y = ps_yp.tile([128, SC], f32, tag="y")
                        for e in range(HPC):
                            nc.tensor.matmul(
                                ps_y[:],
                                oT[:, e, 128 * m : 128 * (m + 1)],
                                woT_sb[:, e, fj],
                                start=e == 0,
                                stop=e == HPC - 1,
                            )
                        y_sb = y_pool.tile([128, SC], f16)
                        # DVE copies: ACT is exp-saturated during attention
                        nc.vector.tensor_copy(y_sb[:], ps_y[:])
                        dma_engs[(m * NCH + fc) % 3].dma_start(
                            y[128 * m : 128 * (m + 1), fj], y_sb[:]
                        )

            for c in range(NCH):
                scj = slice(SC * c, SC * (c + 1))
                nblk = 4 * (c + 1)
                for h in range(HPC):
                    ps_o = ps_attn_o.tile([128, SC], f32, tag="o")
                    ps_r = ps_attn_r.tile([1, SC], f32, tag="r")
                    q_sl = qkT[:, h, scj]
                    es_tiles = {}
                    # software-pipelined 2 deep: emit scores(jk+2) before
                    # pv(jk) so PE covers the exp+mask latency of block jk
                    PD = 2

                    # diagonal blocks with offset 128*d have their first
                    # 128*d score columns fully masked: skip them in
                    # scores/exp/mask/PV/rowsum (keep moving dim >= 128)
                    def _lo(jk):
                        d = jk - 4 * c
                        if d < 1:
                            return 0
                        return min(128 * d, 384)

                    for jk in range(nblk + PD):
                        if jk < nblk:
                            lo = _lo(jk)
                            ps_s = ps_attn_s.tile([128, SC], f32, tag="s")
                            nc.tensor.matmul(
                                ps_s[:, lo:SC],
                                qkT[:, HPC, 128 * jk : 128 * (jk + 1)],
                                q_sl[:, lo:SC],
                                start=True,
                                stop=True,
                            )
                            es = es_pool.tile([128, SC], f16)
                            nc.scalar.activation(
                                es[:, lo:SC],
                                ps_s[:, lo:SC],
                                mybir.ActivationFunctionType.Exp,
                                scale=SCALE,
                            )
                            diag = jk - 4 * c
                            if diag >= 0:
                                off = 128 * diag
                                nc.vector.tensor_mul(
                                    es[:, lo:SC],
                                    es[:, lo:SC],
                                    mask_sb[:, SC - off + lo : 1024 - off],
                                )
                            es_tiles[jk] = es
                        if jk >= PD:
                            pj = jk - PD
                            es = es_tiles.pop(pj)
                            lo = _lo(pj)
                            st, sp = pj == 0, pj == nblk - 1
                            nc.tensor.matmul(
                                ps_o[:, lo:SC],
                                v_sb[:, pj, :],
                                es[:, lo:SC],
                                start=st,
                                stop=sp,
                            )
                            nc.tensor.matmul(
                                ps_r[:, lo:SC],
                                ones_sb[:],
                                es[:, lo:SC],
                                start=st,
                                stop=sp,
                            )
                    # normalize: oT[:, h, chunk] = ps_o * (1/rowsum); fast
                    # approx reciprocal (~18 bits) straight from PSUM
                    r1 = r_pool.tile([1, SC], f32)
                    nc.vector.reciprocal_approx_fast(r1[:], ps_r[:])
                    rb = rb_pool.tile([128, SC], f32)
                    nc.gpsimd.partition_broadcast(rb[:], r1[:])
                    nc.vector.tensor_mul(oT[:, h, scj], ps_o[:], rb[:])

                if c == 0:
                    # chunk-3 RoPE (only read by attn(c3)) runs now that
                    # attn(c0)'s DVE work is queued ahead of it
                    for args in deferred_rope:
                        rope_muls(*args)
                # wo runs one chunk behind attention so oT/normalize for
                # chunk c-1 is long complete when its matmuls issue
                if c >= 1:
                    emit_wo(c - 1)
            emit_wo(NCH - 1)

    nc.compile()
    return nc


def _get_module():
    if "nc" not in _CACHE:
        _CACHE["nc"] = _build_module()
    return _CACHE["nc"]


def _prep_inputs(x, freqs_cos, freqs_sin, wq, wk, wv, wo):
    """Host-side shard/layout prep. Returns per-core input maps."""
    perm = np.concatenate([np.arange(0, 128, 2), np.arange(1, 128, 2)])
    f16 = np.float16
    xT = [np.ascontiguousarray(np.asarray(x[b]).T).astype(f16) for b in range(B)]
    cos_t = np.asarray(freqs_cos).T
    sin_t = np.asarray(freqs_sin).T
    cs = np.concatenate([cos_t, sin_t], axis=0).astype(f16)  # [128, S]
    csw = np.concatenate([sin_t, cos_t], axis=0).astype(f16)  # [128, S]
    # big causal mask: mask[p, g] = 1.0 iff p <= g - 512
    p_idx = np.arange(128)[:, None]
    g_idx = np.arange(1024)[None, :]
    mask = (p_idx <= g_idx - 512).astype(f16)

    wq, wk, wv, wo = (np.asarray(a) for a in (wq, wk, wv, wo))
    in_maps = []
    for i in range(N_CORES):
        b, g = i // 4, i % 4
        wq_i = wq[512 * g : 512 * (g + 1)]  # [512, D] heads 4g..4g+3
        blocks = [wq_i[128 * h + perm] for h in range(HPC)]
        blocks.append(wk[128 * g : 128 * (g + 1)][perm])  # [128, D]
        blocks.append(wv[128 * g : 128 * (g + 1)])  # [128, D] (not permuted)
        # wT[ko, p, f, e] = blocks[f].T[128*ko + p, e]  (ko-major for chunk-0
        # streaming)
        wt = np.stack([np.ascontiguousarray(blk.T) for blk in blocks])  # [f, D, e]
        wT_i = np.ascontiguousarray(
            wt.reshape(NF, NKO, 128, 128).transpose(1, 2, 0, 3)
        ).astype(f16)  # [ko, p, f, e]
        woT_i = np.ascontiguousarray(wo[:, 512 * g : 512 * (g + 1)].T).astype(f16)
        in_maps.append(
            {
                "xT": xT[b],
                "wT": wT_i,
                "woT": woT_i,
                "cs": cs,
                "csw": csw,
                "mask": mask,
                "onesd": np.ones((128, 1), dtype=f16),
            }
        )
    return in_maps


def kernel(x, freqs_cos, freqs_sin, wq, wk, wv, wo):
    from concourse.bass_utils import run_bass_kernel_spmd

    nc = _get_module()
    in_maps = _prep_inputs(x, freqs_cos, freqs_sin, wq, wk, wv, wo)
    res = run_bass_kernel_spmd(nc, in_maps, list(range(N_CORES)))
    out = np.zeros((B, S, D), dtype=np.float32)
    for i in range(N_CORES):
        out[i // 4] += res.results[i]["y"].astype(np.float32)
    return out


if __name__ == "__main__":
    nc = _get_module()
    print(
        "instructions:",
        sum(len(blk.instructions) for blk in nc.m.functions[0].blocks),
    )
